# revision 16
# baseline (speedup 1.0000x reference)
"""Trainium2 Bass kernel for nn_Decoder (2-layer LSTM + local attention + vocab
projection), sharded across 8 NeuronCores.

Sharding strategy:
  - LSTM gate rows (4H) are sharded over cores (each core computes its 128-wide
    h-chunk of every gate); full h1/h2 are rebuilt with AllGathers.
  - Attention: p_t/score partials are contraction-sharded and combined with one
    AllGather + local rank-sum; context/Wcomb are contraction-sharded and
    combined the same way.
  - Vocab projection: Wout is column(V)-sharded; log-softmax uses a tiny
    AllGather of per-core (max, sumexp) stats.
All weights are pre-laid-out on the host so every DMA is contiguous.
"""

import numpy as np
import ml_dtypes

V = 50257
E = 1024
H = 1024
L = 256
D = 10
NCORES = 8
SHARD = 6283          # ceil(V / 8); last core real width is V - 7*SHARD = 6276
VS = 6400             # padded per-core vocab width = 50 tiles of 128
NVT = VS // 128       # 50 v-tiles
BF16 = ml_dtypes.bfloat16

_PROGRAM = None


# --------------------------------------------------------------------------
# host-side input sharding
# --------------------------------------------------------------------------

def _prep_in_maps(inputs):
    f32 = np.float32
    a = {k: np.asarray(v) for k, v in inputs.items()}

    tok = int(np.asarray(a["input_tok"]).reshape(-1)[0])
    emb_row = a["emb"][tok].astype(f32).reshape(-1)            # [1024]
    htt_in = a["h_t_tilde"].astype(f32).reshape(-1)            # [1024]
    x = np.concatenate([emb_row, htt_in])                      # [2048]
    xcol = np.ascontiguousarray(x.reshape(16, 128).T)          # [128,16]
    h00col = np.ascontiguousarray(a["h0"][0, 0].reshape(8, 128).T)
    h01col = np.ascontiguousarray(a["h0"][1, 0].reshape(8, 128).T)

    idxrow = np.arange(L, dtype=f32).reshape(1, L)
    id8 = np.eye(8, dtype=f32)
    id128 = np.eye(128, dtype=f32)
    one1 = np.ones((1, 1), f32)
    ones128 = np.ones((1, 128), f32)
    ones8 = np.ones((8, 1), f32)
    onescol = np.ones((128, 1), f32)
    bcombcol = np.ascontiguousarray(a["bcomb"].astype(f32).reshape(8, 128).T)

    def blob_rhs(Wm, nchunk):
        # Wm [rows(512), K] -> [128, nchunk*512] with chunk c = Wm[:, c*128:(c+1)*128].T
        Kdim = Wm.shape[1]
        assert Kdim == nchunk * 128
        return np.ascontiguousarray(
            Wm.T.reshape(nchunk, 128, Wm.shape[0]).transpose(1, 0, 2).reshape(128, -1)
        )

    in_maps = []
    for r in range(NCORES):
        sl = slice(r * 128, (r + 1) * 128)
        # local gate order [i, f, o, g]; torch order is i,f,g,o
        rows = np.concatenate(
            [g * H + np.arange(r * 128, (r + 1) * 128) for g in (0, 1, 3, 2)]
        )
        big0 = np.concatenate([a["Wih0"][rows], a["Whh0"][rows]], axis=1)  # [512,3072]
        w0 = blob_rhs(big0, 24)
        b0 = (a["bih0"] + a["bhh0"])[rows].astype(f32).reshape(1, 512)
        big1 = np.concatenate([a["Wih1"][rows], a["Whh1"][rows]], axis=1)  # [512,2048]
        w1 = blob_rhs(big1, 16)
        b1 = (a["bih1"] + a["bhh1"])[rows].astype(f32).reshape(1, 512)

        wp = blob_rhs(np.ascontiguousarray(a["Wp"][sl]), 8)                # [128,1024]
        wdotrow = a["wdot"][:, sl].astype(f32).reshape(1, 128)
        Wb = a["Wbil"][0][:, sl]                                           # [1024,128]
        wbil = np.ascontiguousarray(
            Wb.reshape(8, 128, 128).transpose(1, 0, 2).reshape(128, 1024)
        )
        hsT = np.ascontiguousarray(a["h_s"][:, 0, sl].T)                   # [128,256]
        hsnat = np.ascontiguousarray(
            np.concatenate([a["h_s"][c * 128:(c + 1) * 128, 0, sl] for c in range(2)],
                           axis=1)
        )                                                                  # [128,256]
        Wc = a["Wcomb"]
        wcomb = np.ascontiguousarray(
            np.concatenate([Wc[:, sl].T, Wc[:, H + r * 128:H + (r + 1) * 128].T],
                           axis=1)
        )                                                                  # [128,2048]

        vbase = r * SHARD
        realw = min(SHARD, V - vbase)
        Wsl = np.zeros((VS, H), f32)
        Wsl[:realw] = a["Wout"][vbase:vbase + realw]
        woutT = np.ascontiguousarray(
            Wsl.T.astype(ml_dtypes.float8_e4m3)).view(np.uint8)            # [1024,6400]
        bb = np.full((VS,), -1e30, f32)
        bb[:realw] = a["bout"][vbase:vbase + realw]
        boutcol = np.ascontiguousarray(bb.reshape(NVT, 128).T)             # [128,50]

        in_maps.append({
            "xcol": xcol, "h00col": h00col, "h01col": h01col,
            "c00row": np.ascontiguousarray(a["c0"][0, 0, sl]).reshape(1, 128),
            "c01row": np.ascontiguousarray(a["c0"][1, 0, sl]).reshape(1, 128),
            "w0": w0, "b0": b0, "w1": w1, "b1": b1,
            "wp": wp, "wdotrow": wdotrow, "wbil": wbil,
            "hsT": hsT, "hsnat": hsnat, "wcomb": wcomb, "bcombcol": bcombcol,
            "idxrow": idxrow, "id8": id8, "id128": id128, "one1": one1,
            "ones128": ones128, "ones8": ones8, "onescol": onescol,
            "woutT": woutT, "boutcol": boutcol,
        })
    return in_maps


def _assemble(results):
    f32 = np.float32
    y = np.empty((V,), f32)
    for r in range(NCORES):
        vbase = r * SHARD
        realw = min(SHARD, V - vbase)
        t = np.asarray(results[r]["ylog"], f32)            # [128, 50]
        y[vbase:vbase + realw] = t.T.reshape(-1)[:realw]
    h_new = np.stack([
        np.concatenate([np.asarray(results[r]["h1out"]).reshape(-1) for r in range(NCORES)]),
        np.concatenate([np.asarray(results[r]["h2out"]).reshape(-1) for r in range(NCORES)]),
    ]).reshape(2, 1, H).astype(f32)
    c_new = np.stack([
        np.concatenate([np.asarray(results[r]["c1out"]).reshape(-1) for r in range(NCORES)]),
        np.concatenate([np.asarray(results[r]["c2out"]).reshape(-1) for r in range(NCORES)]),
    ]).reshape(2, 1, H).astype(f32)
    htt = np.ascontiguousarray(
        np.asarray(results[0]["httout"], f32).T).reshape(1, 1, H)
    a_t = np.asarray(results[0]["atout"], f32).reshape(L, 1, 1)
    p_t = np.asarray(results[0]["ptout"], f32).reshape(())
    return (y.reshape(1, 1, V), h_new, c_new, htt, a_t, p_t)


# --------------------------------------------------------------------------
# device program
# --------------------------------------------------------------------------

def _build_program():
    import concourse.bass as bass
    import concourse.tile as tile
    from concourse import bacc, mybir

    f32 = mybir.dt.float32
    f32r = mybir.dt.float32r
    bf16 = mybir.dt.bfloat16
    AF = mybir.ActivationFunctionType
    ALU = mybir.AluOpType
    AX = mybir.AxisListType

    nc = bacc.Bacc("TRN2", target_bir_lowering=False, debug=False,
                   num_devices=NCORES)

    def din(name, shape, dt=f32):
        return nc.dram_tensor(name, list(shape), dt, kind="ExternalInput").ap()

    def dout(name, shape, dt=f32):
        return nc.dram_tensor(name, list(shape), dt, kind="ExternalOutput").ap()

    xcol = din("xcol", (128, 16), f32r); h00col = din("h00col", (128, 8), f32r)
    h01col = din("h01col", (128, 8), f32r)
    c00row = din("c00row", (1, 128)); c01row = din("c01row", (1, 128))
    w0_i = din("w0", (128, 24 * 512), f32r); b0_i = din("b0", (1, 512))
    w1_i = din("w1", (128, 16 * 512), f32r); b1_i = din("b1", (1, 512))
    wp_i = din("wp", (128, 8 * 128), f32r); wdot_i = din("wdotrow", (1, 128))
    wbil_i = din("wbil", (128, 8 * 128), f32r)
    hsT_i = din("hsT", (128, 256), f32r); hsnat_i = din("hsnat", (128, 256), f32r)
    wcomb_i = din("wcomb", (128, 2048), f32r); bcomb_i = din("bcombcol", (128, 8))
    idx_i = din("idxrow", (1, L)); id8_i = din("id8", (8, 8))
    id128_i = din("id128", (128, 128))
    one1_i = din("one1", (1, 1)); ones128_i = din("ones128", (1, 128))
    ones8_i = din("ones8", (8, 1)); onescol_i = din("onescol", (128, 1))
    woutT_i = din("woutT", (H, VS), mybir.dt.uint8); bout_i = din("boutcol", (128, NVT))

    h1out = dout("h1out", (1, 128)); c1out = dout("c1out", (1, 128))
    h2out = dout("h2out", (1, 128)); c2out = dout("c2out", (1, 128))
    httout = dout("httout", (128, 8)); atout = dout("atout", (1, L))
    ptout = dout("ptout", (1, 1)); ylog = dout("ylog", (128, NVT))

    with tile.TileContext(nc) as tc:
        with tc.tile_pool(name="wts", bufs=1) as wts, \
             tc.tile_pool(name="wout", bufs=1) as woutp, \
             tc.tile_pool(name="sm", bufs=1) as sm, \
             tc.tile_pool(name="ps", bufs=4, space="PSUM") as ps, \
             tc.tile_pool(name="pw", bufs=1, space="PSUM") as pw, \
             tc.tile_pool(name="py", bufs=2, space="PSUM") as py, \
             tc.tile_pool(name="dram", bufs=1, space="DRAM") as dram:

            from concourse.bass import _add_dep_helper
            dma_groups = {}

            def load(pool, ap_in, shape, dt=f32, tag=None, group=None):
                t = pool.tile(list(shape), dt, tag=tag)
                ins = nc.sync.dma_start(t[:], ap_in)
                if group is not None:
                    dma_groups.setdefault(group, []).append(ins)
                return t

            def order_groups(earlier, later):
                for d in dma_groups.get(later, []):
                    for e in dma_groups.get(earlier, []):
                        _add_dep_helper(d.ins, e.ins, True, "dma-order")

            # ---- small inputs (critical path first) ----
            xc = load(sm, xcol, (128, 16), f32r, tag="xc")
            h00 = load(sm, h00col, (128, 8), f32r, tag="h00")
            h01 = load(sm, h01col, (128, 8), f32r, tag="h01")
            c00 = load(sm, c00row, (1, 128), tag="c00")
            c01 = load(sm, c01row, (1, 128), tag="c01")
            b0r = load(sm, b0_i, (1, 512), tag="b0r")
            b1r = load(sm, b1_i, (1, 512), tag="b1r")
            id8 = load(sm, id8_i, (8, 8), tag="id8")
            one1 = load(sm, one1_i, (1, 1), tag="one1")
            ones128 = load(sm, ones128_i, (1, 128), tag="ones128")
            ones8 = load(sm, ones8_i, (8, 1), tag="ones8")
            onescol = load(sm, onescol_i, (128, 1), tag="onescol")
            id128 = load(sm, id128_i, (128, 128), tag="id128")
            idxr = load(sm, idx_i, (1, L), tag="idxr")
            wdotr = load(sm, wdot_i, (1, 128), tag="wdotr")
            bcombc = load(sm, bcomb_i, (128, 8), tag="bcombc")
            boutc = load(sm, bout_i, (128, NVT), tag="boutc")

            # ---- weights (order = DMA priority) ----
            w0 = wts.tile([128, 24 * 512], f32r, tag="w0")
            for part in range(4):
                sl6 = slice(part * 6 * 512, (part + 1) * 6 * 512)
                dma_groups.setdefault("g0", []).append(
                    nc.sync.dma_start(w0[:, sl6], w0_i[:, sl6]))
            w1 = wts.tile([128, 16 * 512], f32r, tag="w1")
            for part in range(2):
                sl8 = slice(part * 8 * 512, (part + 1) * 8 * 512)
                dma_groups.setdefault("g1", []).append(
                    nc.sync.dma_start(w1[:, sl8], w1_i[:, sl8]))
            wp = load(wts, wp_i, (128, 8 * 128), f32r, tag="wp", group="g2")
            wbil = load(wts, wbil_i, (128, 8 * 128), f32r, tag="wbil", group="g2")
            hsT = load(wts, hsT_i, (128, 256), f32r, tag="hsT", group="g2")
            hsnat = load(wts, hsnat_i, (128, 256), f32r, tag="hsnat", group="g2")
            wcomb = load(wts, wcomb_i, (128, 2048), f32r, tag="wcomb", group="g2")
            wo_tiles = []
            for c in range(8):
                wo = woutp.tile([128, VS], mybir.dt.uint8, tag=f"wo{c}")
                dma_groups.setdefault("g3", []).append(
                    nc.sync.dma_start(wo[:], woutT_i[c * 128:(c + 1) * 128, :]))
                wo_tiles.append(wo)
            order_groups("g0", "g1")
            order_groups("g1", "g2")
            order_groups("g2", "g3")

            def sigmoid_row(dst, src_ap, width=None):
                nc.scalar.activation(dst, src_ap, AF.Sigmoid)

            def tanh_row(dst, src_ap, width=None):
                nc.scalar.activation(dst, src_ap, AF.Tanh)

            def gates_math(g, brow, crow):
                gb = sm.tile([1, 512], f32, tag="gb")
                nc.vector.tensor_tensor(gb[:], g[:], brow[:], ALU.add)
                sig = sm.tile([1, 384], f32, tag="sig")
                sigmoid_row(sig[:], gb[0:1, 0:384], 384)
                tg = sm.tile([1, 128], f32, tag="tg")
                tanh_row(tg[:], gb[0:1, 384:512], 128)
                t1 = sm.tile([1, 128], f32, tag="t1")
                nc.vector.tensor_tensor(t1[:], sig[0:1, 128:256], crow[:], ALU.mult)
                t2 = sm.tile([1, 128], f32, tag="t2")
                nc.vector.tensor_tensor(t2[:], sig[0:1, 0:128], tg[:], ALU.mult)
                cnew = sm.tile([1, 128], f32, tag="cnew")
                nc.vector.tensor_tensor(cnew[:], t1[:], t2[:], ALU.add)
                tc2 = sm.tile([1, 128], f32, tag="tc2")
                tanh_row(tc2[:], cnew[:], 128)
                hnew = sm.tile([1, 128], f32, tag="hnew")
                nc.vector.tensor_tensor(hnew[:], sig[0:1, 256:384], tc2[:], ALU.mult)
                return hnew, cnew

            def lstm_layer(wblob, nk_x, lx, lh, brow, crow):
                # gates psum [1,512]; order [i,f,o,g]
                g = ps.tile([1, 512], f32, tag="ps")
                nki = nk_x + 8
                for c in range(nki):
                    lhsT = (lx[:, c:c + 1] if c < nk_x else lh[:, c - nk_x:c - nk_x + 1])
                    nc.tensor.matmul(
                        g[:], lhsT, wblob[:, c * 512:(c + 1) * 512],
                        start=(c == 0), stop=(c == nki - 1))
                return gates_math(g, brow, crow)

            def allgather(row_tile, width, tag):
                gin = dram.tile([1, width], f32, tag=tag + "i")
                gout = dram.tile([NCORES, width], f32, tag=tag + "o")
                nc.sync.dma_start(gin[:], row_tile[:])
                nc.gpsimd.collective_compute(
                    "AllGather", mybir.AluOpType.bypass,
                    replica_groups=[list(range(NCORES))],
                    ins=[gin.opt()], outs=[gout.opt()])
                allv = sm.tile([NCORES, width], f32, tag=tag + "s")
                nc.sync.dma_start(allv[:], gout[:])
                return allv

            def cols_from_allgather(allv, tag):
                # [8,128] -> [128,8] via PE transpose
                pt = ps.tile([128, 8], f32, tag="ps")
                nc.tensor.transpose(pt[:], allv[:], id8[:])
                colt = sm.tile([128, 8], f32r, tag=tag)
                nc.vector.tensor_copy(colt[:], pt[:])
                return colt

            # ================= LSTM =================
            h1row, c1row = lstm_layer(w0, 16, xc, h00, b0r, c00)
            nc.sync.dma_start(h1out, h1row[:])
            nc.sync.dma_start(c1out, c1row[:])
            # Whh1 half of L1 gates doesn't need h1 -> overlap with AG1
            g1 = ps.tile([1, 512], f32, tag="ps")
            for c in range(8):
                nc.tensor.matmul(g1[:], h01[:, c:c + 1],
                                 w1[:, (8 + c) * 512:(9 + c) * 512],
                                 start=(c == 0), stop=False)
            h1all = allgather(h1row, 128, "ag1")
            h1col = cols_from_allgather(h1all, "h1col")
            for c in range(8):
                nc.tensor.matmul(g1[:], h1col[:, c:c + 1],
                                 w1[:, c * 512:(c + 1) * 512],
                                 start=False, stop=(c == 7))
            h2row, c2row = gates_math(g1, b1r, c01)
            nc.sync.dma_start(h2out, h2row[:])
            nc.sync.dma_start(c2out, c2row[:])
            h2all = allgather(h2row, 128, "ag2")
            h2col = cols_from_allgather(h2all, "h2col")

            # ================= attention scalars =================
            # p_t partial: tanh(h2 @ Wp_chunk.T) . wdot_chunk
            tp = ps.tile([1, 128], f32, tag="ps")
            for c in range(8):
                nc.tensor.matmul(tp[:], h2col[:, c:c + 1], wp[:, c * 128:(c + 1) * 128],
                                 start=(c == 0), stop=(c == 7))
            tpt = sm.tile([1, 128], f32, tag="tpt")
            tanh_row(tpt[:], tp[:], 128)
            pp = sm.tile([1, 128], f32, tag="pp")
            nc.vector.tensor_tensor(pp[:], tpt[:], wdotr[:], ALU.mult)

            # q chunk [128,1]
            q = ps.tile([128, 1], f32, tag="ps")
            for c in range(8):
                nc.tensor.matmul(q[:], wbil[:, c * 128:(c + 1) * 128].bitcast(f32),
                                 h2col[:, c:c + 1].bitcast(f32),
                                 start=(c == 0), stop=(c == 7))
            qs = sm.tile([128, 1], f32r, tag="qs")
            nc.vector.tensor_copy(qs[:], q[:])
            # score partial [1,256]
            scp = ps.tile([1, 256], f32, tag="ps")
            nc.tensor.matmul(scp[:], qs[:], hsT[:], start=True, stop=True)
            scrow = sm.tile([1, L + 1], f32, tag="scrow")
            nc.vector.tensor_copy(scrow[0:1, 0:L], scp[:])
            nc.vector.tensor_reduce(scrow[0:1, L:L + 1], pp[:], AX.X, ALU.add)

            scall = allgather(scrow, L + 1, "ag3")
            scf = ps.tile([1, L + 1], f32, tag="ps")
            nc.tensor.matmul(scf[:], ones8[:], scall[:], start=True, stop=True)

            # p_t = 256 * sigmoid(z)
            sg = sm.tile([1, 1], f32, tag="sg")
            nc.scalar.activation(sg[:], scf[0:1, L:L + 1], AF.Exp, scale=-1.0)
            nc.vector.tensor_scalar_add(sg[:], sg[:], 1.0)
            nc.vector.reciprocal(sg[:], sg[:])
            ptr = sm.tile([1, 1], f32, tag="ptr")
            nc.vector.tensor_scalar_mul(ptr[:], sg[:], 256.0)
            nc.sync.dma_start(ptout, ptr[:])

            # window mask
            lo_t = sm.tile([1, 1], f32, tag="lo_t")
            nc.vector.tensor_scalar_add(lo_t[:], ptr[:], -10.5)
            hi_t = sm.tile([1, 1], f32, tag="hi_t")
            nc.vector.tensor_scalar_add(hi_t[:], ptr[:], 10.5)
            m1 = sm.tile([1, L], f32, tag="m1")
            nc.vector.tensor_scalar(m1[:], idxr[:], lo_t[:], None, ALU.is_ge)
            m2 = sm.tile([1, L], f32, tag="m2")
            nc.vector.tensor_scalar(m2[:], idxr[:], hi_t[:], None, ALU.is_le)
            mask = sm.tile([1, L], f32, tag="mask")
            nc.vector.tensor_tensor(mask[:], m1[:], m2[:], ALU.mult)
            win = sm.tile([1, 1], f32, tag="win")
            nc.vector.tensor_reduce(win[:], mask[:], AX.X, ALU.add)
            invw = sm.tile([1, 1], f32, tag="invw")
            nc.vector.reciprocal(invw[:], win[:])

            # masked softmax over scores
            moff = sm.tile([1, L], f32, tag="moff")
            nc.vector.tensor_scalar(moff[:], mask[:], 1e30, -1e30, ALU.mult, ALU.add)
            smsk = sm.tile([1, L], f32, tag="smsk")
            nc.vector.tensor_tensor(smsk[:], scf[0:1, 0:L], mask[:], ALU.mult)
            nc.vector.tensor_tensor(smsk[:], smsk[:], moff[:], ALU.add)
            mx = sm.tile([1, 1], f32, tag="mx")
            nc.vector.tensor_reduce(mx[:], smsk[:], AX.X, ALU.max)
            nmx = sm.tile([1, 1], f32, tag="nmx")
            nc.vector.tensor_scalar_mul(nmx[:], mx[:], -1.0)
            ex = sm.tile([1, L], f32, tag="ex")
            S = sm.tile([1, 1], f32, tag="S")
            nc.scalar.activation(ex[:], smsk[:], AF.Exp, bias=nmx[:], accum_out=S[:])
            invS = sm.tile([1, 1], f32, tag="invS")
            nc.vector.reciprocal(invS[:], S[:])
            arow = sm.tile([1, L], f32, tag="arow")
            nc.vector.tensor_scalar(arow[:], ex[:], invS[:], None, ALU.mult)

            # a_t = a * exp(mask * (idx - p_t)^2 / 25)
            dd = sm.tile([1, L], f32, tag="dd")
            nc.vector.tensor_scalar(dd[:], idxr[:], ptr[:], 0.2, ALU.subtract, ALU.mult)
            dsq = sm.tile([1, L], f32, tag="dsq")
            nc.vector.tensor_tensor(dsq[:], dd[:], dd[:], ALU.mult)
            nc.vector.tensor_tensor(dsq[:], dsq[:], mask[:], ALU.mult)
            eexp = sm.tile([1, L], f32, tag="eexp")
            nc.scalar.activation(eexp[:], dsq[:], AF.Exp)
            atrow = sm.tile([1, L], f32, tag="atrow")
            nc.vector.tensor_tensor(atrow[:], arow[:], eexp[:], ALU.mult)
            nc.sync.dma_start(atout, atrow[:])
            asc = sm.tile([1, L], f32, tag="asc")
            nc.vector.tensor_scalar(asc[:], atrow[:], invw[:], None, ALU.mult)

            # context chunk [128,1]
            pa = ps.tile([128, 2], f32, tag="ps")
            for c in range(2):
                nc.tensor.matmul(pa[:, c:c + 1], asc[0:1, c * 128:(c + 1) * 128],
                                 one1[:], start=True, stop=True)
            acol = sm.tile([128, 2], f32r, tag="acol")
            nc.vector.tensor_copy(acol[:], pa[:])
            pctx = ps.tile([128, 1], f32, tag="ps")
            for c in range(2):
                nc.tensor.matmul(pctx[:],
                                 hsnat[:, c * 128:(c + 1) * 128].bitcast(f32),
                                 acol[:, c:c + 1].bitcast(f32),
                                 start=(c == 0), stop=(c == 1))
            ctx = sm.tile([128, 1], f32r, tag="ctx")
            nc.vector.tensor_copy(ctx[:], pctx[:])

            # h_t chunk of this core as [128,1]
            ph2 = ps.tile([128, 1], f32, tag="ps")
            nc.tensor.matmul(ph2[:], h2row[:], one1[:], start=True, stop=True)
            h2self = sm.tile([128, 1], f32r, tag="h2self")
            nc.vector.tensor_copy(h2self[:], ph2[:])

            # Wcomb partials: [1,1024] = ctx @ A + h2chunk @ B
            prerow = sm.tile([1, 1024], f32, tag="prerow")
            for n in range(2):
                pcat = pw.tile([1, 512], f32, tag="pw")
                nc.tensor.matmul(pcat[:],
                                 ctx[:], wcomb[:, n * 512:(n + 1) * 512],
                                 start=True, stop=False)
                nc.tensor.matmul(pcat[:],
                                 h2self[:],
                                 wcomb[:, 1024 + n * 512:1024 + (n + 1) * 512],
                                 start=False, stop=True)
                nc.vector.tensor_copy(prerow[0:1, n * 512:(n + 1) * 512], pcat[:])

            preall = allgather(prerow, 1024, "ag4")
            pprec = ps.tile([128, 8], f32, tag="ps")
            for c in range(8):
                nc.tensor.matmul(pprec[:, c:c + 1],
                                 preall[:, c * 128:(c + 1) * 128], ones8[:],
                                 start=True, stop=True)
            prec = sm.tile([128, 8], f32, tag="prec")
            nc.vector.tensor_tensor(prec[:], pprec[:], bcombc[:], ALU.add)
            httcol = sm.tile([128, 8], f32, tag="httcol")
            nc.scalar.activation(httcol[:], prec[:], AF.Exp, scale=-2.0)
            nc.vector.tensor_scalar_add(httcol[:], httcol[:], 1.0)
            nc.vector.reciprocal(httcol[:], httcol[:])
            nc.vector.tensor_scalar(httcol[:], httcol[:], 2.0, -1.0,
                                    ALU.mult, ALU.add)
            nc.sync.dma_start(httout, httcol[:])
            httf8 = sm.tile([128, 8], mybir.dt.float8e4, tag="httf8")
            nc.vector.tensor_copy(httf8[:], httcol[:])

            # ================= vocab projection =================
            acc = sm.tile([128, NVT], f32, tag="acc")
            f8 = mybir.dt.float8e4
            for c in range(8):
                wo = wo_tiles[c]
                pyt = py.tile([128, NVT], f32, tag="py")
                for j in range(NVT):
                    nc.tensor.matmul(pyt[:, j:j + 1],
                                     wo[:, j * 128:(j + 1) * 128].bitcast(f8),
                                     httf8[:, c:c + 1], start=True, stop=True)
                if c == 0:
                    nc.vector.tensor_tensor(acc[:], pyt[:], boutc[:], ALU.add)
                else:
                    nc.vector.tensor_tensor(acc[:], acc[:], pyt[:], ALU.add)

            # local log-softmax stats
            mloc = sm.tile([128, 1], f32, tag="mloc")
            nc.vector.tensor_reduce(mloc[:], acc[:], AX.X, ALU.max)
            pmt = ps.tile([1, 128], f32, tag="ps")
            nc.tensor.transpose(pmt[:], mloc[:], id128[:])
            mg = sm.tile([1, 1], f32, tag="mg")
            nc.vector.tensor_reduce(mg[:], pmt[:], AX.X, ALU.max)
            nmg1 = sm.tile([1, 1], f32, tag="nmg1")
            nc.vector.tensor_scalar_mul(nmg1[:], mg[:], -1.0)
            pb = ps.tile([128, 1], f32, tag="ps")
            nc.tensor.matmul(pb[:], ones128[:], nmg1[:], start=True, stop=True)
            nmcol = sm.tile([128, 1], f32, tag="nmcol")
            nc.vector.tensor_copy(nmcol[:], pb[:])
            es = sm.tile([128, NVT], f32, tag="es")
            srow = sm.tile([128, 1], f32, tag="srow")
            nc.scalar.activation(es[:], acc[:], AF.Exp, bias=nmcol[:],
                                 accum_out=srow[:])
            psl = ps.tile([1, 1], f32, tag="ps")
            nc.tensor.matmul(psl[:], srow[:], onescol[:], start=True, stop=True)
            stats = sm.tile([1, 2], f32, tag="stats")
            nc.vector.tensor_copy(stats[0:1, 0:1], mg[:])
            nc.vector.tensor_copy(stats[0:1, 1:2], psl[:])

            stall = allgather(stats, 2, "ag5")
            pm8 = ps.tile([1, 8], f32, tag="ps")
            nc.tensor.transpose(pm8[:], stall[:, 0:1], id8[:])
            nmg = sm.tile([1, 1], f32, tag="nmg")
            nc.vector.tensor_reduce(nmg[:], pm8[:], AX.X, ALU.max)
            nc.vector.tensor_scalar_mul(nmg[:], nmg[:], -1.0)
            pnm8 = ps.tile([8, 1], f32, tag="ps")
            nc.tensor.matmul(pnm8[:], ones128[0:1, 0:8], nmg[:],
                             start=True, stop=True)
            nm8 = sm.tile([8, 1], f32, tag="nm8")
            nc.vector.tensor_copy(nm8[:], pnm8[:])
            e8 = sm.tile([8, 1], f32, tag="e8")
            nc.scalar.activation(e8[:], stall[:, 0:1], AF.Exp, bias=nm8[:])
            prod8 = sm.tile([8, 1], f32, tag="prod8")
            nc.vector.tensor_tensor(prod8[:], e8[:], stall[:, 1:2], ALU.mult)
            psg = ps.tile([1, 1], f32, tag="ps")
            nc.tensor.matmul(psg[:], prod8[:], ones8[:], start=True, stop=True)
            lng = sm.tile([1, 1], f32, tag="lng")
            nc.scalar.activation(lng[:], psg[:], AF.Ln)
            shift = sm.tile([1, 1], f32, tag="shift")
            nc.vector.tensor_tensor(shift[:], lng[:], nmg[:], ALU.subtract)
            psh = ps.tile([128, 1], f32, tag="ps")
            nc.tensor.matmul(psh[:], ones128[:], shift[:], start=True, stop=True)
            shcol = sm.tile([128, 1], f32, tag="shcol")
            nc.vector.tensor_copy(shcol[:], psh[:])
            yfin = sm.tile([128, NVT], f32, tag="yfin")
            nc.vector.tensor_scalar(yfin[:], acc[:], shcol[:], None, ALU.subtract)
            nc.sync.dma_start(ylog, yfin[:])

    nc.compile()
    return nc


def _get_program():
    global _PROGRAM
    if _PROGRAM is None:
        _PROGRAM = _build_program()
    return _PROGRAM


def run(inputs, trace=False, tmpdir=None):
    from concourse.bass_utils import run_bass_kernel_spmd
    nc = _get_program()
    in_maps = _prep_in_maps(inputs)
    res = run_bass_kernel_spmd(nc, in_maps, core_ids=list(range(NCORES)),
                               trace=trace, tmpdir=tmpdir)
    return _assemble(res.results), res


def kernel(**inputs):
    outs, _ = run(inputs)
    return outs


# revision 17
# speedup vs baseline: 1.0364x; 1.0364x over previous
"""Trainium2 Bass kernel for nn_Decoder (2-layer LSTM + local attention + vocab
projection), sharded across 8 NeuronCores.

Sharding strategy:
  - LSTM gate rows (4H) are sharded over cores (each core computes its 128-wide
    h-chunk of every gate); full h1/h2 are rebuilt with AllGathers.
  - Attention: p_t/score partials are contraction-sharded and combined with one
    AllGather + local rank-sum; context/Wcomb are contraction-sharded and
    combined the same way.
  - Vocab projection: Wout is column(V)-sharded; log-softmax uses a tiny
    AllGather of per-core (max, sumexp) stats.
All weights are pre-laid-out on the host so every DMA is contiguous.
"""

import numpy as np
import ml_dtypes

V = 50257
E = 1024
H = 1024
L = 256
D = 10
NCORES = 8
SHARD = 6283          # ceil(V / 8); last core real width is V - 7*SHARD = 6276
VS = 6400             # padded per-core vocab width = 50 tiles of 128
NVT = VS // 128       # 50 v-tiles
BF16 = ml_dtypes.bfloat16

_PROGRAM = None


# --------------------------------------------------------------------------
# host-side input sharding
# --------------------------------------------------------------------------

def _prep_in_maps(inputs):
    f32 = np.float32
    a = {k: np.asarray(v) for k, v in inputs.items()}

    tok = int(np.asarray(a["input_tok"]).reshape(-1)[0])
    emb_row = a["emb"][tok].astype(f32).reshape(-1)            # [1024]
    htt_in = a["h_t_tilde"].astype(f32).reshape(-1)            # [1024]
    x = np.concatenate([emb_row, htt_in])                      # [2048]
    xcol = np.ascontiguousarray(x.reshape(16, 128).T)          # [128,16]
    h00col = np.ascontiguousarray(a["h0"][0, 0].reshape(8, 128).T)
    h01col = np.ascontiguousarray(a["h0"][1, 0].reshape(8, 128).T)

    idxrow = np.arange(L, dtype=f32).reshape(1, L)
    id8 = np.eye(8, dtype=f32)
    id128 = np.eye(128, dtype=f32)
    one1 = np.ones((1, 1), f32)
    ones128 = np.ones((1, 128), f32)
    ones8 = np.ones((8, 1), f32)
    onescol = np.ones((128, 1), f32)
    bcombcol = np.ascontiguousarray(a["bcomb"].astype(f32).reshape(8, 128).T)

    def blob_rhs(Wm, nchunk):
        # Wm [rows(512), K] -> [128, nchunk*512] with chunk c = Wm[:, c*128:(c+1)*128].T
        Kdim = Wm.shape[1]
        assert Kdim == nchunk * 128
        return np.ascontiguousarray(
            Wm.T.reshape(nchunk, 128, Wm.shape[0]).transpose(1, 0, 2).reshape(128, -1)
        )

    in_maps = []
    for r in range(NCORES):
        sl = slice(r * 128, (r + 1) * 128)
        # local gate order [i, f, o, g]; torch order is i,f,g,o
        rows = np.concatenate(
            [g * H + np.arange(r * 128, (r + 1) * 128) for g in (0, 1, 3, 2)]
        )
        big0 = np.concatenate([a["Wih0"][rows], a["Whh0"][rows]], axis=1)  # [512,3072]
        w0 = blob_rhs(big0, 24)
        b0 = (a["bih0"] + a["bhh0"])[rows].astype(f32).reshape(1, 512)
        big1 = np.concatenate([a["Wih1"][rows], a["Whh1"][rows]], axis=1)  # [512,2048]
        w1 = blob_rhs(big1, 16)
        b1 = (a["bih1"] + a["bhh1"])[rows].astype(f32).reshape(1, 512)

        wp = blob_rhs(np.ascontiguousarray(a["Wp"][sl]), 8)                # [128,1024]
        wdotrow = a["wdot"][:, sl].astype(f32).reshape(1, 128)
        Wb = a["Wbil"][0][:, sl]                                           # [1024,128]
        wbil = np.ascontiguousarray(
            Wb.reshape(8, 128, 128).transpose(1, 0, 2).reshape(128, 1024)
        )
        hsT = np.ascontiguousarray(a["h_s"][:, 0, sl].T)                   # [128,256]
        hsnat = np.ascontiguousarray(
            np.concatenate([a["h_s"][c * 128:(c + 1) * 128, 0, sl] for c in range(2)],
                           axis=1)
        )                                                                  # [128,256]
        Wc = a["Wcomb"]
        wcomb = np.ascontiguousarray(
            np.concatenate([Wc[:, sl].T, Wc[:, H + r * 128:H + (r + 1) * 128].T],
                           axis=1)
        )                                                                  # [128,2048]

        vbase = r * SHARD
        realw = min(SHARD, V - vbase)
        Wsl = np.zeros((VS, H), f32)
        Wsl[:realw] = a["Wout"][vbase:vbase + realw]
        woutT = np.ascontiguousarray(
            Wsl.T.astype(ml_dtypes.float8_e4m3)).view(np.uint8)            # [1024,6400]
        bb = np.full((VS,), -1e30, f32)
        bb[:realw] = a["bout"][vbase:vbase + realw]
        boutcol = np.ascontiguousarray(bb.reshape(NVT, 128).T)             # [128,50]

        in_maps.append({
            "xcol": xcol, "h00col": h00col, "h01col": h01col,
            "c00row": np.ascontiguousarray(a["c0"][0, 0, sl]).reshape(1, 128),
            "c01row": np.ascontiguousarray(a["c0"][1, 0, sl]).reshape(1, 128),
            "w0": w0, "b0": b0, "w1": w1, "b1": b1,
            "wp": wp, "wdotrow": wdotrow, "wbil": wbil,
            "hsT": hsT, "hsnat": hsnat, "wcomb": wcomb, "bcombcol": bcombcol,
            "idxrow": idxrow, "id8": id8, "id128": id128, "one1": one1,
            "ones128": ones128, "ones8": ones8, "onescol": onescol,
            "woutT": woutT, "boutcol": boutcol,
        })
    return in_maps


def _assemble(results):
    f32 = np.float32
    y = np.empty((V,), f32)
    for r in range(NCORES):
        vbase = r * SHARD
        realw = min(SHARD, V - vbase)
        t = np.asarray(results[r]["ylog"], f32)            # [128, 50]
        y[vbase:vbase + realw] = t.T.reshape(-1)[:realw]
    h_new = np.stack([
        np.concatenate([np.asarray(results[r]["h1out"]).reshape(-1) for r in range(NCORES)]),
        np.concatenate([np.asarray(results[r]["h2out"]).reshape(-1) for r in range(NCORES)]),
    ]).reshape(2, 1, H).astype(f32)
    c_new = np.stack([
        np.concatenate([np.asarray(results[r]["c1out"]).reshape(-1) for r in range(NCORES)]),
        np.concatenate([np.asarray(results[r]["c2out"]).reshape(-1) for r in range(NCORES)]),
    ]).reshape(2, 1, H).astype(f32)
    htt = np.ascontiguousarray(
        np.asarray(results[0]["httout"], f32).T).reshape(1, 1, H)
    a_t = np.asarray(results[0]["atout"], f32).reshape(L, 1, 1)
    p_t = np.asarray(results[0]["ptout"], f32).reshape(())
    return (y.reshape(1, 1, V), h_new, c_new, htt, a_t, p_t)


# --------------------------------------------------------------------------
# device program
# --------------------------------------------------------------------------

def _build_program():
    import concourse.bass as bass
    import concourse.tile as tile
    from concourse import bacc, mybir

    f32 = mybir.dt.float32
    f32r = mybir.dt.float32r
    bf16 = mybir.dt.bfloat16
    AF = mybir.ActivationFunctionType
    ALU = mybir.AluOpType
    AX = mybir.AxisListType

    nc = bacc.Bacc("TRN2", target_bir_lowering=False, debug=False,
                   num_devices=NCORES)

    def din(name, shape, dt=f32):
        return nc.dram_tensor(name, list(shape), dt, kind="ExternalInput").ap()

    def dout(name, shape, dt=f32):
        return nc.dram_tensor(name, list(shape), dt, kind="ExternalOutput").ap()

    xcol = din("xcol", (128, 16), f32r); h00col = din("h00col", (128, 8), f32r)
    h01col = din("h01col", (128, 8), f32r)
    c00row = din("c00row", (1, 128)); c01row = din("c01row", (1, 128))
    w0_i = din("w0", (128, 24 * 512), f32r); b0_i = din("b0", (1, 512))
    w1_i = din("w1", (128, 16 * 512), f32r); b1_i = din("b1", (1, 512))
    wp_i = din("wp", (128, 8 * 128), f32r); wdot_i = din("wdotrow", (1, 128))
    wbil_i = din("wbil", (128, 8 * 128), f32r)
    hsT_i = din("hsT", (128, 256), f32r); hsnat_i = din("hsnat", (128, 256), f32r)
    wcomb_i = din("wcomb", (128, 2048), f32r); bcomb_i = din("bcombcol", (128, 8))
    idx_i = din("idxrow", (1, L)); id8_i = din("id8", (8, 8))
    id128_i = din("id128", (128, 128))
    one1_i = din("one1", (1, 1)); ones128_i = din("ones128", (1, 128))
    ones8_i = din("ones8", (8, 1)); onescol_i = din("onescol", (128, 1))
    woutT_i = din("woutT", (H, VS), mybir.dt.uint8); bout_i = din("boutcol", (128, NVT))

    h1out = dout("h1out", (1, 128)); c1out = dout("c1out", (1, 128))
    h2out = dout("h2out", (1, 128)); c2out = dout("c2out", (1, 128))
    httout = dout("httout", (128, 8)); atout = dout("atout", (1, L))
    ptout = dout("ptout", (1, 1)); ylog = dout("ylog", (128, NVT))

    with tile.TileContext(nc) as tc:
        with tc.tile_pool(name="wts", bufs=1) as wts, \
             tc.tile_pool(name="wout", bufs=1) as woutp, \
             tc.tile_pool(name="sm", bufs=1) as sm, \
             tc.tile_pool(name="ps", bufs=4, space="PSUM") as ps, \
             tc.tile_pool(name="pw", bufs=1, space="PSUM") as pw, \
             tc.tile_pool(name="py", bufs=3, space="PSUM") as py, \
             tc.tile_pool(name="dram", bufs=1, space="DRAM") as dram:

            from concourse.bass import _add_dep_helper
            dma_groups = {}

            def load(pool, ap_in, shape, dt=f32, tag=None, group=None):
                t = pool.tile(list(shape), dt, tag=tag)
                ins = nc.sync.dma_start(t[:], ap_in)
                if group is not None:
                    dma_groups.setdefault(group, []).append(ins)
                return t

            def order_groups(earlier, later):
                for d in dma_groups.get(later, []):
                    for e in dma_groups.get(earlier, []):
                        _add_dep_helper(d.ins, e.ins, True, "dma-order")

            # ---- small inputs (critical path first) ----
            xc = load(sm, xcol, (128, 16), f32r, tag="xc")
            h00 = load(sm, h00col, (128, 8), f32r, tag="h00")
            h01 = load(sm, h01col, (128, 8), f32r, tag="h01")
            c00 = load(sm, c00row, (1, 128), tag="c00")
            c01 = load(sm, c01row, (1, 128), tag="c01")
            b0r = load(sm, b0_i, (1, 512), tag="b0r")
            b1r = load(sm, b1_i, (1, 512), tag="b1r")
            id8 = load(sm, id8_i, (8, 8), tag="id8")
            one1 = load(sm, one1_i, (1, 1), tag="one1")
            ones128 = load(sm, ones128_i, (1, 128), tag="ones128")
            ones8 = load(sm, ones8_i, (8, 1), tag="ones8")
            onescol = load(sm, onescol_i, (128, 1), tag="onescol")
            id128 = load(sm, id128_i, (128, 128), tag="id128")
            idxr = load(sm, idx_i, (1, L), tag="idxr")
            wdotr = load(sm, wdot_i, (1, 128), tag="wdotr")
            bcombc = load(sm, bcomb_i, (128, 8), tag="bcombc")
            boutc = load(sm, bout_i, (128, NVT), tag="boutc")

            # ---- weights (order = DMA priority) ----
            w0 = wts.tile([128, 24 * 512], f32r, tag="w0")
            for part in range(4):
                sl6 = slice(part * 6 * 512, (part + 1) * 6 * 512)
                dma_groups.setdefault("g0", []).append(
                    nc.sync.dma_start(w0[:, sl6], w0_i[:, sl6]))
            w1 = wts.tile([128, 16 * 512], f32r, tag="w1")
            for part in range(2):
                sl8 = slice(part * 8 * 512, (part + 1) * 8 * 512)
                dma_groups.setdefault("g1", []).append(
                    nc.sync.dma_start(w1[:, sl8], w1_i[:, sl8]))
            wp = load(wts, wp_i, (128, 8 * 128), f32r, tag="wp", group="g2")
            wbil = load(wts, wbil_i, (128, 8 * 128), f32r, tag="wbil", group="g2")
            hsT = load(wts, hsT_i, (128, 256), f32r, tag="hsT", group="g2")
            hsnat = load(wts, hsnat_i, (128, 256), f32r, tag="hsnat", group="g2")
            wcomb = load(wts, wcomb_i, (128, 2048), f32r, tag="wcomb", group="g2")
            wo_tiles = []
            for c in range(8):
                wo = woutp.tile([128, VS], mybir.dt.uint8, tag=f"wo{c}")
                dma_groups.setdefault("g3", []).append(
                    nc.sync.dma_start(wo[:], woutT_i[c * 128:(c + 1) * 128, :]))
                wo_tiles.append(wo)
            order_groups("g0", "g1")
            order_groups("g1", "g2")
            order_groups("g2", "g3")

            def sigmoid_row(dst, src_ap, width=None):
                nc.scalar.activation(dst, src_ap, AF.Sigmoid)

            def tanh_row(dst, src_ap, width=None):
                nc.scalar.activation(dst, src_ap, AF.Tanh)

            def gates_math(g, brow, crow):
                gb = sm.tile([1, 512], f32, tag="gb")
                nc.vector.tensor_tensor(gb[:], g[:], brow[:], ALU.add)
                sig = sm.tile([1, 384], f32, tag="sig")
                sigmoid_row(sig[:], gb[0:1, 0:384], 384)
                tg = sm.tile([1, 128], f32, tag="tg")
                tanh_row(tg[:], gb[0:1, 384:512], 128)
                t1 = sm.tile([1, 128], f32, tag="t1")
                nc.vector.tensor_tensor(t1[:], sig[0:1, 128:256], crow[:], ALU.mult)
                t2 = sm.tile([1, 128], f32, tag="t2")
                nc.vector.tensor_tensor(t2[:], sig[0:1, 0:128], tg[:], ALU.mult)
                cnew = sm.tile([1, 128], f32, tag="cnew")
                nc.vector.tensor_tensor(cnew[:], t1[:], t2[:], ALU.add)
                tc2 = sm.tile([1, 128], f32, tag="tc2")
                tanh_row(tc2[:], cnew[:], 128)
                hnew = sm.tile([1, 128], f32, tag="hnew")
                nc.vector.tensor_tensor(hnew[:], sig[0:1, 256:384], tc2[:], ALU.mult)
                return hnew, cnew

            def lstm_layer(wblob, nk_x, lx, lh, brow, crow):
                # gates psum [1,512]; order [i,f,o,g]
                g = ps.tile([1, 512], f32, tag="ps")
                nki = nk_x + 8
                for c in range(nki):
                    lhsT = (lx[:, c:c + 1] if c < nk_x else lh[:, c - nk_x:c - nk_x + 1])
                    nc.tensor.matmul(
                        g[:], lhsT, wblob[:, c * 512:(c + 1) * 512],
                        start=(c == 0), stop=(c == nki - 1))
                return gates_math(g, brow, crow)

            def allgather(row_tile, width, tag):
                gin = dram.tile([1, width], f32, tag=tag + "i")
                gout = dram.tile([NCORES, width], f32, tag=tag + "o")
                nc.sync.dma_start(gin[:], row_tile[:])
                nc.gpsimd.collective_compute(
                    "AllGather", mybir.AluOpType.bypass,
                    replica_groups=[list(range(NCORES))],
                    ins=[gin.opt()], outs=[gout.opt()])
                allv = sm.tile([NCORES, width], f32, tag=tag + "s")
                nc.sync.dma_start(allv[:], gout[:])
                return allv

            def cols_from_allgather(allv, tag):
                # [8,128] -> [128,8] via PE transpose
                pt = ps.tile([128, 8], f32, tag="ps")
                nc.tensor.transpose(pt[:], allv[:], id8[:])
                colt = sm.tile([128, 8], f32r, tag=tag)
                nc.vector.tensor_copy(colt[:], pt[:])
                return colt

            # ================= LSTM =================
            h1row, c1row = lstm_layer(w0, 16, xc, h00, b0r, c00)
            nc.sync.dma_start(h1out, h1row[:])
            nc.sync.dma_start(c1out, c1row[:])
            # Whh1 half of L1 gates doesn't need h1 -> overlap with AG1
            g1 = ps.tile([1, 512], f32, tag="ps")
            for c in range(8):
                nc.tensor.matmul(g1[:], h01[:, c:c + 1],
                                 w1[:, (8 + c) * 512:(9 + c) * 512],
                                 start=(c == 0), stop=False)
            h1all = allgather(h1row, 128, "ag1")
            h1col = cols_from_allgather(h1all, "h1col")
            for c in range(8):
                nc.tensor.matmul(g1[:], h1col[:, c:c + 1],
                                 w1[:, c * 512:(c + 1) * 512],
                                 start=False, stop=(c == 7))
            h2row, c2row = gates_math(g1, b1r, c01)
            nc.sync.dma_start(h2out, h2row[:])
            nc.sync.dma_start(c2out, c2row[:])
            # own h_t chunk as [128,1]: only needs h2row -> overlap with AG2
            ph2 = ps.tile([128, 1], f32, tag="ps")
            nc.tensor.matmul(ph2[:], h2row[:], one1[:], start=True, stop=True)
            h2self = sm.tile([128, 1], f32r, tag="h2self")
            nc.vector.tensor_copy(h2self[:], ph2[:])
            h2all = allgather(h2row, 128, "ag2")
            h2col = cols_from_allgather(h2all, "h2col")

            # ================= attention scalars =================
            # p_t partial: tanh(h2 @ Wp_chunk.T) . wdot_chunk
            tp = ps.tile([1, 128], f32, tag="ps")
            for c in range(8):
                nc.tensor.matmul(tp[:], h2col[:, c:c + 1], wp[:, c * 128:(c + 1) * 128],
                                 start=(c == 0), stop=(c == 7))
            tpt = sm.tile([1, 128], f32, tag="tpt")
            tanh_row(tpt[:], tp[:], 128)
            pp = sm.tile([1, 128], f32, tag="pp")
            nc.vector.tensor_tensor(pp[:], tpt[:], wdotr[:], ALU.mult)

            # q chunk [128,1]
            q = ps.tile([128, 1], f32, tag="ps")
            for c in range(8):
                nc.tensor.matmul(q[:], wbil[:, c * 128:(c + 1) * 128].bitcast(f32),
                                 h2col[:, c:c + 1].bitcast(f32),
                                 start=(c == 0), stop=(c == 7))
            qs = sm.tile([128, 1], f32r, tag="qs")
            nc.vector.tensor_copy(qs[:], q[:])
            # score partial [1,256]
            scp = ps.tile([1, 256], f32, tag="ps")
            nc.tensor.matmul(scp[:], qs[:], hsT[:], start=True, stop=True)
            scrow = sm.tile([1, L + 1], f32, tag="scrow")
            nc.vector.tensor_copy(scrow[0:1, 0:L], scp[:])
            nc.vector.tensor_reduce(scrow[0:1, L:L + 1], pp[:], AX.X, ALU.add)

            scall = allgather(scrow, L + 1, "ag3")
            scf = ps.tile([1, L + 1], f32, tag="ps")
            nc.tensor.matmul(scf[:], ones8[:], scall[:], start=True, stop=True)

            # p_t = 256 * sigmoid(z)
            sg = sm.tile([1, 1], f32, tag="sg")
            nc.scalar.activation(sg[:], scf[0:1, L:L + 1], AF.Exp, scale=-1.0)
            nc.vector.tensor_scalar_add(sg[:], sg[:], 1.0)
            nc.vector.reciprocal(sg[:], sg[:])
            ptr = sm.tile([1, 1], f32, tag="ptr")
            nc.vector.tensor_scalar_mul(ptr[:], sg[:], 256.0)
            nc.sync.dma_start(ptout, ptr[:])

            # window mask
            lo_t = sm.tile([1, 1], f32, tag="lo_t")
            nc.vector.tensor_scalar_add(lo_t[:], ptr[:], -10.5)
            hi_t = sm.tile([1, 1], f32, tag="hi_t")
            nc.vector.tensor_scalar_add(hi_t[:], ptr[:], 10.5)
            m1 = sm.tile([1, L], f32, tag="m1")
            nc.vector.tensor_scalar(m1[:], idxr[:], lo_t[:], None, ALU.is_ge)
            m2 = sm.tile([1, L], f32, tag="m2")
            nc.vector.tensor_scalar(m2[:], idxr[:], hi_t[:], None, ALU.is_le)
            mask = sm.tile([1, L], f32, tag="mask")
            nc.vector.tensor_tensor(mask[:], m1[:], m2[:], ALU.mult)
            win = sm.tile([1, 1], f32, tag="win")
            nc.vector.tensor_reduce(win[:], mask[:], AX.X, ALU.add)
            invw = sm.tile([1, 1], f32, tag="invw")
            nc.vector.reciprocal(invw[:], win[:])

            # masked softmax over scores
            moff = sm.tile([1, L], f32, tag="moff")
            nc.vector.tensor_scalar(moff[:], mask[:], 1e30, -1e30, ALU.mult, ALU.add)
            smsk = sm.tile([1, L], f32, tag="smsk")
            nc.vector.tensor_tensor(smsk[:], scf[0:1, 0:L], mask[:], ALU.mult)
            nc.vector.tensor_tensor(smsk[:], smsk[:], moff[:], ALU.add)
            mx = sm.tile([1, 1], f32, tag="mx")
            nc.vector.tensor_reduce(mx[:], smsk[:], AX.X, ALU.max)
            nmx = sm.tile([1, 1], f32, tag="nmx")
            nc.vector.tensor_scalar_mul(nmx[:], mx[:], -1.0)
            ex = sm.tile([1, L], f32, tag="ex")
            S = sm.tile([1, 1], f32, tag="S")
            nc.scalar.activation(ex[:], smsk[:], AF.Exp, bias=nmx[:], accum_out=S[:])
            invS = sm.tile([1, 1], f32, tag="invS")
            nc.vector.reciprocal(invS[:], S[:])
            arow = sm.tile([1, L], f32, tag="arow")
            nc.vector.tensor_scalar(arow[:], ex[:], invS[:], None, ALU.mult)

            # a_t = a * exp(mask * (idx - p_t)^2 / 25)
            dd = sm.tile([1, L], f32, tag="dd")
            nc.vector.tensor_scalar(dd[:], idxr[:], ptr[:], 0.2, ALU.subtract, ALU.mult)
            dsq = sm.tile([1, L], f32, tag="dsq")
            nc.vector.tensor_tensor(dsq[:], dd[:], dd[:], ALU.mult)
            nc.vector.tensor_tensor(dsq[:], dsq[:], mask[:], ALU.mult)
            eexp = sm.tile([1, L], f32, tag="eexp")
            nc.scalar.activation(eexp[:], dsq[:], AF.Exp)
            atrow = sm.tile([1, L], f32, tag="atrow")
            nc.vector.tensor_tensor(atrow[:], arow[:], eexp[:], ALU.mult)
            nc.sync.dma_start(atout, atrow[:])
            asc = sm.tile([1, L], f32, tag="asc")
            nc.vector.tensor_scalar(asc[:], atrow[:], invw[:], None, ALU.mult)

            # context chunk [128,1]
            pa = ps.tile([128, 2], f32, tag="ps")
            for c in range(2):
                nc.tensor.matmul(pa[:, c:c + 1], asc[0:1, c * 128:(c + 1) * 128],
                                 one1[:], start=True, stop=True)
            acol = sm.tile([128, 2], f32r, tag="acol")
            nc.vector.tensor_copy(acol[:], pa[:])
            pctx = ps.tile([128, 1], f32, tag="ps")
            for c in range(2):
                nc.tensor.matmul(pctx[:],
                                 hsnat[:, c * 128:(c + 1) * 128].bitcast(f32),
                                 acol[:, c:c + 1].bitcast(f32),
                                 start=(c == 0), stop=(c == 1))
            ctx = sm.tile([128, 1], f32r, tag="ctx")
            nc.vector.tensor_copy(ctx[:], pctx[:])

            # Wcomb partials: [1,1024] = ctx @ A + h2chunk @ B
            prerow = sm.tile([1, 1024], f32, tag="prerow")
            for n in range(2):
                pcat = pw.tile([1, 512], f32, tag="pw")
                nc.tensor.matmul(pcat[:],
                                 ctx[:], wcomb[:, n * 512:(n + 1) * 512],
                                 start=True, stop=False)
                nc.tensor.matmul(pcat[:],
                                 h2self[:],
                                 wcomb[:, 1024 + n * 512:1024 + (n + 1) * 512],
                                 start=False, stop=True)
                nc.vector.tensor_copy(prerow[0:1, n * 512:(n + 1) * 512], pcat[:])

            preall = allgather(prerow, 1024, "ag4")
            pprec = ps.tile([128, 8], f32, tag="ps")
            for c in range(8):
                nc.tensor.matmul(pprec[:, c:c + 1],
                                 preall[:, c * 128:(c + 1) * 128], ones8[:],
                                 start=True, stop=True)
            prec = sm.tile([128, 8], f32, tag="prec")
            nc.vector.tensor_tensor(prec[:], pprec[:], bcombc[:], ALU.add)
            httcol = sm.tile([128, 8], f32, tag="httcol")
            nc.scalar.activation(httcol[:], prec[:], AF.Exp, scale=-2.0)
            nc.vector.tensor_scalar_add(httcol[:], httcol[:], 1.0)
            nc.vector.reciprocal(httcol[:], httcol[:])
            nc.vector.tensor_scalar(httcol[:], httcol[:], 2.0, -1.0,
                                    ALU.mult, ALU.add)
            nc.sync.dma_start(httout, httcol[:])
            httf8 = sm.tile([128, 8], mybir.dt.float8e4, tag="httf8")
            nc.vector.tensor_copy(httf8[:], httcol[:])

            # ================= vocab projection =================
            acc = sm.tile([128, NVT], f32, tag="acc")
            f8 = mybir.dt.float8e4
            for c in range(8):
                wo = wo_tiles[c]
                pyt = py.tile([128, NVT], f32, tag="py")
                for j in range(NVT):
                    nc.tensor.matmul(pyt[:, j:j + 1],
                                     wo[:, j * 128:(j + 1) * 128].bitcast(f8),
                                     httf8[:, c:c + 1], start=True, stop=True)
                if c == 0:
                    nc.vector.tensor_tensor(acc[:], pyt[:], boutc[:], ALU.add)
                else:
                    nc.vector.tensor_tensor(acc[:], acc[:], pyt[:], ALU.add)

            # local log-softmax stats
            mloc = sm.tile([128, 1], f32, tag="mloc")
            nc.vector.tensor_reduce(mloc[:], acc[:], AX.X, ALU.max)
            pmt = ps.tile([1, 128], f32, tag="ps")
            nc.tensor.transpose(pmt[:], mloc[:], id128[:])
            mg = sm.tile([1, 1], f32, tag="mg")
            nc.vector.tensor_reduce(mg[:], pmt[:], AX.X, ALU.max)
            nmg1 = sm.tile([1, 1], f32, tag="nmg1")
            nc.vector.tensor_scalar_mul(nmg1[:], mg[:], -1.0)
            pb = ps.tile([128, 1], f32, tag="ps")
            nc.tensor.matmul(pb[:], ones128[:], nmg1[:], start=True, stop=True)
            nmcol = sm.tile([128, 1], f32, tag="nmcol")
            nc.vector.tensor_copy(nmcol[:], pb[:])
            es = sm.tile([128, NVT], f32, tag="es")
            srow = sm.tile([128, 1], f32, tag="srow")
            nc.scalar.activation(es[:], acc[:], AF.Exp, bias=nmcol[:],
                                 accum_out=srow[:])
            psl = ps.tile([1, 1], f32, tag="ps")
            nc.tensor.matmul(psl[:], srow[:], onescol[:], start=True, stop=True)
            stats = sm.tile([1, 2], f32, tag="stats")
            nc.vector.tensor_copy(stats[0:1, 0:1], mg[:])
            nc.vector.tensor_copy(stats[0:1, 1:2], psl[:])

            stall = allgather(stats, 2, "ag5")
            pm8 = ps.tile([1, 8], f32, tag="ps")
            nc.tensor.transpose(pm8[:], stall[:, 0:1], id8[:])
            nmg = sm.tile([1, 1], f32, tag="nmg")
            nc.vector.tensor_reduce(nmg[:], pm8[:], AX.X, ALU.max)
            nc.vector.tensor_scalar_mul(nmg[:], nmg[:], -1.0)
            pnm8 = ps.tile([8, 1], f32, tag="ps")
            nc.tensor.matmul(pnm8[:], ones128[0:1, 0:8], nmg[:],
                             start=True, stop=True)
            nm8 = sm.tile([8, 1], f32, tag="nm8")
            nc.vector.tensor_copy(nm8[:], pnm8[:])
            e8 = sm.tile([8, 1], f32, tag="e8")
            nc.scalar.activation(e8[:], stall[:, 0:1], AF.Exp, bias=nm8[:])
            prod8 = sm.tile([8, 1], f32, tag="prod8")
            nc.vector.tensor_tensor(prod8[:], e8[:], stall[:, 1:2], ALU.mult)
            psg = ps.tile([1, 1], f32, tag="ps")
            nc.tensor.matmul(psg[:], prod8[:], ones8[:], start=True, stop=True)
            lng = sm.tile([1, 1], f32, tag="lng")
            nc.scalar.activation(lng[:], psg[:], AF.Ln)
            shift = sm.tile([1, 1], f32, tag="shift")
            nc.vector.tensor_tensor(shift[:], lng[:], nmg[:], ALU.subtract)
            psh = ps.tile([128, 1], f32, tag="ps")
            nc.tensor.matmul(psh[:], ones128[:], shift[:], start=True, stop=True)
            shcol = sm.tile([128, 1], f32, tag="shcol")
            nc.vector.tensor_copy(shcol[:], psh[:])
            yfin = sm.tile([128, NVT], f32, tag="yfin")
            nc.vector.tensor_scalar(yfin[:], acc[:], shcol[:], None, ALU.subtract)
            nc.sync.dma_start(ylog, yfin[:])

    nc.compile()
    return nc


def _get_program():
    global _PROGRAM
    if _PROGRAM is None:
        _PROGRAM = _build_program()
    return _PROGRAM


def run(inputs, trace=False, tmpdir=None):
    from concourse.bass_utils import run_bass_kernel_spmd
    nc = _get_program()
    in_maps = _prep_in_maps(inputs)
    res = run_bass_kernel_spmd(nc, in_maps, core_ids=list(range(NCORES)),
                               trace=trace, tmpdir=tmpdir)
    return _assemble(res.results), res


def kernel(**inputs):
    outs, _ = run(inputs)
    return outs
